# revision 15
# baseline (speedup 1.0000x reference)
"""DSS (diagonal state space) layer on 8 Trainium2 NeuronCores.

Per h channel: y_conv[b,l] = sum_{m<=l} k[m] u[b,l-m], k[m] = Re(sum_n c_n w_n^m),
computed without FFT via chunked modal decomposition (Q=128, J=16 chunks):
  intra-chunk lower-tri Toeplitz matmul (D-skip folded into k[0]),
  per-chunk modal summaries S_n[i] = sum_q w^{Q-1-q} u[iQ+q] (matmul),
  complex linear scan over chunks (DVE), apply matmul with Re/Im(c w^{t+1}).
Then exact GELU (ACT), AllToAll h->batch reshard, 768x768 output linear.
Core c returns output batch c; conv is h-sharded (96 h/core, all batches).
"""

import sys

sys.path.insert(0, "/opt/trn_rl_repo")

import numpy as np
import ml_dtypes

B, H, N, L, C = 8, 768, 64, 2048, 1
P = 8            # cores
HL = H // P      # 96 h per core
Q = 128          # chunk length
J = L // Q       # 16 chunks
BJ = B * J       # 128 matmul columns per h
HP = HL // 2     # 48 h-pairs per core
NG = 2           # h-groups (pipelining scan vs matmuls)
GHP = HP // NG   # 24 h-pairs per group
GW = GHP * B     # 192 scan columns per group (per re/im half)
SW = HP * B      # 384 scan columns total (per re/im half)

_BF16 = ml_dtypes.bfloat16


def _host_precompute(u, log_dt, Lambda, W, D, out_w, out_b):
    """Numpy: DSS coefficients + per-core NEFF input arrays."""
    dt = np.exp(log_dt.astype(np.float64))                       # (H,2)
    lam = Lambda[:, 0].astype(np.float64) + 1j * Lambda[:, 1].astype(np.float64)
    dtl = dt[:, 0:1] * lam.real[None, :] + 1j * (dt[:, 1:2] * lam.imag[None, :])
    w = np.exp(dtl)                                              # (H,N)
    num = np.exp(dtl) - 1.0
    den = np.exp(dtl * L) - 1.0
    x = den * lam[None, :]
    recip = np.conj(x) / (x * np.conj(x) + 1e-7)
    c = (W[0, :, :, 0].astype(np.float64) + 1j * W[0, :, :, 1].astype(np.float64))
    c = c * num * recip                                          # (H,N)

    tpow = w[:, :, None] ** np.arange(0, Q + 1)[None, None, :]   # (H,N,Q+1)
    k = np.real(np.einsum("hn,hnt->ht", c, tpow[:, :, :Q]))      # (H,Q)
    kp = k.copy()
    kp[:, 0] += D[0, :].astype(np.float64)
    wQ = w ** Q                                                  # (H,N)
    cw1 = c[:, :, None] * tpow[:, :, 1:Q + 1]                    # c*w^{t+1} (H,N,Q)

    # t0t[h][s][t] = kp[h, t-s] (t>=s else 0)
    d_i = np.arange(Q)[None, :] - np.arange(Q)[:, None]          # (s,t)
    t0t = np.where(d_i >= 0, kp[:, np.clip(d_i, 0, Q - 1)], 0.0)  # (H,Q,Q)

    # vpk[h][q][n] = Re(w^{Q-1-q}); [q][64+n] = Im(w^{Q-1-q})
    rev = tpow[:, :, Q - 1::-1]                                  # (H,N,Q)
    vpk = np.concatenate([rev.real.transpose(0, 2, 1),
                          rev.imag.transpose(0, 2, 1)], axis=2)  # (H,Q,2N)

    # ppk/mpk packed per h-pair: rows hs*64+n
    ppk = cw1.real.reshape(H // 2, 2 * N, Q)                     # (H/2,128,Q)
    mpk = (-cw1.imag).reshape(H // 2, 2 * N, Q)

    # scan multipliers per core: [128=(hs*64+n), (hp,b)]
    wq_re = wQ.real.reshape(H // 2, 2, N)                        # (hp_all, hs, n)
    wq_im = wQ.imag.reshape(H // 2, 2, N)

    per_core = []
    for cc in range(P):
        h0 = cc * HL
        hp0 = h0 // 2
        # [hs,n] x [hp,b]
        wr = np.repeat(wq_re[hp0:hp0 + HP].transpose(1, 2, 0), B, axis=2)
        wr = wr.reshape(128, SW).astype(np.float32)
        wi = np.repeat(wq_im[hp0:hp0 + HP].transpose(1, 2, 0), B, axis=2)
        wi = wi.reshape(128, SW).astype(np.float32)
        # transpose(1,2,0): (hs, n, hp); repeat b -> (hs, n, hp*B) rows hs*64+n ok

        # u_t[h][q][b*J+j] = u[b, h0+h, j*Q+q]
        uc = u[:, h0:h0 + HL, :].reshape(B, HL, J, Q)
        u_t = uc.transpose(1, 3, 0, 2).reshape(HL, Q, BJ)

        per_core.append(dict(
            u_t=np.ascontiguousarray(u_t).astype(_BF16),
            t0t=np.ascontiguousarray(t0t[h0:h0 + HL]).astype(_BF16),
            vpk=np.ascontiguousarray(vpk[h0:h0 + HL]).astype(_BF16),
            ppk=np.ascontiguousarray(ppk[hp0:hp0 + HP]).astype(_BF16),
            mpk=np.ascontiguousarray(mpk[hp0:hp0 + HP]).astype(_BF16),
            wr=wr, wi=wi,
        ))

    outwT = np.ascontiguousarray(out_w.T).astype(_BF16)          # (u=H, v=H)
    outb = np.ascontiguousarray(
        out_b[:, 0].reshape(H // 128, 128).T).astype(np.float32)  # (128, 6)
    return per_core, outwT, outb


def _build_nc():
    from concourse import bacc
    import concourse.mybir as mybir

    from concourse.tile import TileContext

    f32 = mybir.dt.float32
    bf16 = mybir.dt.bfloat16
    ACT = mybir.ActivationFunctionType
    ALU = mybir.AluOpType

    nc = bacc.Bacc()
    u_t = nc.dram_tensor("u_t", [HL, Q, BJ], bf16, kind="ExternalInput")
    t0t = nc.dram_tensor("t0t", [HL, Q, Q], bf16, kind="ExternalInput")
    vpk = nc.dram_tensor("vpk", [HL, Q, 2 * N], bf16, kind="ExternalInput")
    ppk = nc.dram_tensor("ppk", [HP, 2 * N, Q], bf16, kind="ExternalInput")
    mpk = nc.dram_tensor("mpk", [HP, 2 * N, Q], bf16, kind="ExternalInput")
    wr_d = nc.dram_tensor("wr", [128, SW], f32, kind="ExternalInput")
    wi_d = nc.dram_tensor("wi", [128, SW], f32, kind="ExternalInput")
    outwT = nc.dram_tensor("outwT", [H, H], bf16, kind="ExternalInput")
    outb = nc.dram_tensor("outb", [128, H // 128], f32, kind="ExternalInput")
    y_out = nc.dram_tensor("y", [H, L], f32, kind="ExternalOutput")

    with TileContext(nc) as tc:
        with (
            tc.tile_pool(name="dram", bufs=1, space="DRAM") as dpool,
            tc.tile_pool(name="scan", bufs=1) as spool,
            tc.tile_pool(name="uin", bufs=8) as upool,
            tc.tile_pool(name="wts", bufs=4) as wpool,
            tc.tile_pool(name="tmp", bufs=3) as tpool,
            tc.tile_pool(name="gout", bufs=4) as gpool,
            tc.tile_pool(name="outph", bufs=1) as opool,
            tc.tile_pool(name="ostr", bufs=16) as ospool,
            tc.tile_pool(name="ps_s", bufs=2, space="PSUM") as ps_s,
            tc.tile_pool(name="ps_y", bufs=2, space="PSUM") as ps_y,
            tc.tile_pool(name="ps_o", bufs=4, space="PSUM") as ps_o,
        ):
            # payload layout [b, j, h_local, t]: the gelu-store DMA needs
            # (hh,t) contiguous to stay within the 3-dim DMA limit
            a2a_in = dpool.tile([B, J, HL, Q], bf16)
            a2a_out = dpool.tile([B, J, HL, Q], bf16)

            wr_t = spool.tile([128, SW], f32, tag="wr")
            wi_t = spool.tile([128, SW], f32, tag="wi")
            nc.sync.dma_start(wr_t[:], wr_d[:])
            nc.sync.dma_start(wi_t[:], wi_d[:])

            # scan buffers: free dim = (hp, b, j) — hp-major so the apply
            # matmul lhsT slice [hs*64:, hp*128:(hp+1)*128] is contiguous.
            Sre = spool.tile([128, J * SW], bf16, tag="Sre")
            Sim = spool.tile([128, J * SW], bf16, tag="Sim")
            Zre = spool.tile([128, J * SW], f32, tag="Zre")
            Zim = spool.tile([128, J * SW], f32, tag="Zim")
            Zbre = spool.tile([128, J * SW], bf16, tag="Zbre")
            Zbim = spool.tile([128, J * SW], bf16, tag="Zbim")

            def jslice(t, j, g):
                # [128, hp(GHP), b(B)] view of group g at chunk j
                v = t[:].rearrange("p (hp b j) -> p hp b j", hp=HP, b=B)
                return v[:, g * GHP:(g + 1) * GHP, :, j]

            # ---------------- phase 1: summaries ----------------
            for g in range(NG):
                for hq in range(GHP // 2):  # 4-h blocks
                    ps = ps_s.tile([128, 512], f32)
                    for hp2 in range(2):
                        hp = g * GHP + hq * 2 + hp2
                        for hs in range(2):
                            h = hp * 2 + hs
                            ut = upool.tile([Q, BJ], bf16, tag="u")
                            nc.sync.dma_start(ut[:], u_t[h])
                            vt = wpool.tile([Q, 2 * N], bf16, tag="v")
                            nc.sync.dma_start(vt[:], vpk[h])
                            for ri in range(2):
                                nc.tensor.matmul(
                                    ps[hs * 64:(hs + 1) * 64,
                                       hp2 * 256 + ri * 128:
                                       hp2 * 256 + ri * 128 + 128],
                                    vt[:, ri * 64:(ri + 1) * 64],
                                    ut[:],
                                    start=True, stop=True,
                                )
                    # copy S (4 h) into scan layout
                    hp_base = g * GHP + hq * 2
                    psv = ps[:].rearrange("p (hp2 ri b j) -> p hp2 ri b j",
                                          hp2=2, ri=2, b=B, j=J)
                    for ri, dstt in ((0, Sre), (1, Sim)):
                        dst = dstt[:].rearrange("p (hp b j) -> p hp b j",
                                                hp=HP, b=B)
                        nc.vector.tensor_copy(
                            dst[:, hp_base:hp_base + 2, :, :],
                            psv[:, :, ri, :, :])

            # ---------------- scan (fp32 DVE) ----------------
            for g in range(NG):
                wrg = (wr_t[:].rearrange("p (hp b) -> p hp b", hp=HP)
                       [:, g * GHP:(g + 1) * GHP, :])
                wig = (wi_t[:].rearrange("p (hp b) -> p hp b", hp=HP)
                       [:, g * GHP:(g + 1) * GHP, :])
                nc.vector.memset(jslice(Zre, 0, g), 0.0)
                nc.vector.memset(jslice(Zim, 0, g), 0.0)
                for j in range(1, J):
                    zr_p, zi_p = jslice(Zre, j - 1, g), jslice(Zim, j - 1, g)
                    a = tpool.tile([128, GHP, B], f32, tag="sa")
                    b_ = tpool.tile([128, GHP, B], f32, tag="sb")
                    nc.vector.tensor_mul(a[:], zr_p, wrg)
                    nc.vector.tensor_mul(b_[:], zi_p, wig)
                    nc.vector.tensor_sub(a[:], a[:], b_[:])
                    nc.vector.tensor_add(jslice(Zre, j, g), a[:],
                                         jslice(Sre, j - 1, g))
                    a2 = tpool.tile([128, GHP, B], f32, tag="sc")
                    b2 = tpool.tile([128, GHP, B], f32, tag="sd")
                    nc.vector.tensor_mul(a2[:], zi_p, wrg)
                    nc.vector.tensor_mul(b2[:], zr_p, wig)
                    nc.vector.tensor_add(a2[:], a2[:], b2[:])
                    nc.vector.tensor_add(jslice(Zim, j, g), a2[:],
                                         jslice(Sim, j - 1, g))
                # bf16 cast for apply matmuls (group's hp range: contiguous)
                c0, c1 = g * GHP * B * J, (g + 1) * GHP * B * J
                nc.vector.tensor_copy(Zbre[:, c0:c1], Zre[:, c0:c1])
                nc.vector.tensor_copy(Zbim[:, c0:c1], Zim[:, c0:c1])

            # ---------------- phase 2: intra + apply + gelu ----------------
            for g in range(NG):
                for hq in range(GHP // 2):
                    psy = ps_y.tile([128, 512], f32)
                    for hp2 in range(2):
                        hp = g * GHP + hq * 2 + hp2
                        pt = wpool.tile([2 * N, Q], bf16, tag="p")
                        mt = wpool.tile([2 * N, Q], bf16, tag="m")
                        nc.sync.dma_start(pt[:], ppk[hp])
                        nc.sync.dma_start(mt[:], mpk[hp])
                        for hs in range(2):
                            h = hp * 2 + hs
                            hsub = hp2 * 2 + hs
                            tt = wpool.tile([Q, Q], bf16, tag="t0")
                            nc.sync.dma_start(tt[:], t0t[h])
                            ut = upool.tile([Q, BJ], bf16, tag="u")
                            nc.sync.dma_start(ut[:], u_t[h])
                            out_ap = psy[:, hsub * 128:(hsub + 1) * 128]
                            nc.tensor.matmul(out_ap, ut[:], tt[:],
                                             start=True, stop=False)
                            zre = Zbre[hs * 64:(hs + 1) * 64,
                                       hp * 128:(hp + 1) * 128]
                            zim = Zbim[hs * 64:(hs + 1) * 64,
                                       hp * 128:(hp + 1) * 128]
                            nc.tensor.matmul(out_ap, zre,
                                             pt[hs * 64:(hs + 1) * 64, :],
                                             start=False, stop=False)
                            nc.tensor.matmul(out_ap, zim,
                                             mt[hs * 64:(hs + 1) * 64, :],
                                             start=False, stop=True)
                    gt = gpool.tile([128, 512], bf16, tag="g")
                    nc.scalar.activation(gt[:], psy[:], ACT.Gelu)
                    h_base = (g * GHP + hq * 2) * 2
                    nc.sync.dma_start(
                        a2a_in[:, :, h_base:h_base + 4, :], gt[:])

            # ---------------- AllToAll ----------------
            nc.gpsimd.collective_compute(
                "AllToAll", ALU.bypass,
                replica_groups=[list(range(P))],
                ins=[a2a_in.opt()], outs=[a2a_out.opt()],
            )

            # ---------------- output linear ----------------
            # g_full rows are h_global = d*HL + hh (d = source rank);
            # contract in 8 chunks of K=96, one per source rank.
            MT = H // 128
            g_tiles = []
            for d in range(P):
                gt = opool.tile([HL, L], bf16, tag=f"gf{d}")
                nc.sync.dma_start(
                    gt[:].rearrange("hh (j t) -> hh j t", t=Q),
                    a2a_out[d].rearrange("j hh t -> hh j t"))
                g_tiles.append(gt)
            ob_t = opool.tile([128, MT], f32, tag="ob")
            nc.sync.dma_start(ob_t[:], outb[:])

            FT = L // 512
            for m in range(MT):
                ow_m = []
                for k in range(P):
                    wt = ospool.tile([HL, 128], bf16, tag="ow")
                    nc.sync.dma_start(
                        wt[:], outwT[k * HL:(k + 1) * HL,
                                     m * 128:(m + 1) * 128])
                    ow_m.append(wt)
                for f in range(FT):
                    pso = ps_o.tile([128, 512], f32)
                    for k in range(P):
                        nc.tensor.matmul(
                            pso[:],
                            ow_m[k][:],
                            g_tiles[k][:, f * 512:(f + 1) * 512],
                            start=(k == 0), stop=(k == P - 1),
                        )
                    yt = tpool.tile([128, 512], f32, tag="yo")
                    nc.scalar.activation(yt[:], pso[:], ACT.Identity,
                                         bias=ob_t[:, m:m + 1], scale=1.0)
                    nc.sync.dma_start(
                        y_out[m * 128:(m + 1) * 128, f * 512:(f + 1) * 512],
                        yt[:])

    nc.finalize()
    return nc


_CACHED = {}


def kernel(u, log_dt, Lambda, W, D, out_w, out_b, _trace=False, _trace_cores=None):
    from concourse.bass_utils import run_bass_kernel_spmd

    per_core, outwT, outb = _host_precompute(
        np.asarray(u), np.asarray(log_dt), np.asarray(Lambda), np.asarray(W),
        np.asarray(D), np.asarray(out_w), np.asarray(out_b))

    if "nc" not in _CACHED:
        _CACHED["nc"] = _build_nc()
    nc = _CACHED["nc"]

    in_maps = []
    for c in range(P):
        m = dict(per_core[c])
        m["outwT"] = outwT
        m["outb"] = outb
        in_maps.append(m)

    kw = {}
    if _trace:
        kw = dict(trace=True, trace_cores=(_trace_cores or [0]))
    res = run_bass_kernel_spmd(nc, in_maps, core_ids=list(range(P)), **kw)
    out = np.stack([res.results[c]["y"] for c in range(P)])  # (B, H, L)
    if _trace:
        kernel._last_exec_time_ns = res.exec_time_ns
        kernel._last_result = res
    return out


# revision 19
# speedup vs baseline: 1.7938x; 1.7938x over previous
"""DSS (diagonal state space) layer on 8 Trainium2 NeuronCores.

Per h channel: y_conv[b,l] = sum_{m<=l} k[m] u[b,l-m], k[m] = Re(sum_n c_n w_n^m),
computed without FFT via chunked modal decomposition (Q=128, J=16 chunks):
  intra-chunk lower-tri Toeplitz matmul (D-skip folded into k[0]),
  per-chunk modal summaries S_n[i] = sum_q w^{Q-1-q} u[iQ+q] (matmul),
  complex linear scan over chunks (DVE+GpSimd, fp16), apply matmul with
  Re/Im(c w^{t+1}).
Then exact GELU (ACT), AllToAll h->batch reshard, 768x768 output linear.
Core c returns output batch c; conv is h-sharded (96 h/core, all batches).
All conv matmul data is fp16 (fp32 accumulate in PSUM).
"""

import sys

sys.path.insert(0, "/opt/trn_rl_repo")

import numpy as np

B, H, N, L, C = 8, 768, 64, 2048, 1
P = 8            # cores
HL = H // P      # 96 h per core
Q = 128          # chunk length
J = L // Q       # 16 chunks
BJ = B * J       # 128 matmul columns per h (column order j*B+b, j-major)
HP = HL // 2     # 48 h-pairs per core
NG = 2           # h-groups (pipelining scan vs matmuls)
GHP = HP // NG   # 24 h-pairs per group
HG = HL // 8     # 12 8-h DMA blocks
SCAN_SPLIT = 16  # hp per group on DVE; rest (GHP-SCAN_SPLIT) on GpSimd

F16 = np.float16


def _host_precompute(u, log_dt, Lambda, W, D, out_w, out_b):
    """Numpy: DSS coefficients + per-core NEFF input arrays."""
    dt = np.exp(log_dt.astype(np.float64))                       # (H,2)
    lam = Lambda[:, 0].astype(np.float64) + 1j * Lambda[:, 1].astype(np.float64)
    dtl = dt[:, 0:1] * lam.real[None, :] + 1j * (dt[:, 1:2] * lam.imag[None, :])
    w = np.exp(dtl)                                              # (H,N)
    num = np.exp(dtl) - 1.0
    den = np.exp(dtl * L) - 1.0
    x = den * lam[None, :]
    recip = np.conj(x) / (x * np.conj(x) + 1e-7)
    c = (W[0, :, :, 0].astype(np.float64) + 1j * W[0, :, :, 1].astype(np.float64))
    c = c * num * recip                                          # (H,N)

    tpow = w[:, :, None] ** np.arange(0, Q + 1)[None, None, :]   # (H,N,Q+1)
    k = np.real(np.einsum("hn,hnt->ht", c, tpow[:, :, :Q]))      # (H,Q)
    kp = k.copy()
    kp[:, 0] += D[0, :].astype(np.float64)
    wQ = w ** Q                                                  # (H,N)
    cw1 = c[:, :, None] * tpow[:, :, 1:Q + 1]                    # c*w^{t+1} (H,N,Q)

    # t0t[h][s][t] = kp[h, t-s] (t>=s else 0)
    d_i = np.arange(Q)[None, :] - np.arange(Q)[:, None]          # (s,t)
    t0t = np.where(d_i >= 0, kp[:, np.clip(d_i, 0, Q - 1)], 0.0)  # (H,Q,Q)

    # vpk[h][q][n] = Re(w^{Q-1-q}); [q][64+n] = Im(w^{Q-1-q})
    rev = tpow[:, :, Q - 1::-1]                                  # (H,N,Q)
    vpk = np.concatenate([rev.real.transpose(0, 2, 1),
                          rev.imag.transpose(0, 2, 1)], axis=2)  # (H,Q,2N)

    # ppk/mpk rows hs*64+n per h-pair
    ppk = cw1.real.reshape(H // 2, 2 * N, Q)                     # (H/2,128,Q)
    mpk = (-cw1.imag).reshape(H // 2, 2 * N, Q)

    wq_re = wQ.real.reshape(H // 2, 2, N)                        # (hp_all,hs,n)
    wq_im = wQ.imag.reshape(H // 2, 2, N)

    per_core = []
    for cc in range(P):
        h0 = cc * HL
        hp0 = h0 // 2
        # scan multipliers [128=(hs,n), (hp,b)]
        wr = np.repeat(wq_re[hp0:hp0 + HP].transpose(1, 2, 0), B, axis=2)
        wr = wr.reshape(128, HP * B).astype(F16)
        wi = np.repeat(wq_im[hp0:hp0 + HP].transpose(1, 2, 0), B, axis=2)
        wi = wi.reshape(128, HP * B).astype(F16)

        # u_t8[hg][q][hsub][j*B+b] = u[b, h, j*Q+q],  h = hg*8+hsub
        uc = u[:, h0:h0 + HL, :].reshape(B, HL, J, Q)
        u_t = uc.transpose(1, 3, 2, 0).reshape(HL, Q, BJ)        # (h, q, (j,b))
        u_t8 = u_t.reshape(HG, 8, Q, BJ).transpose(0, 2, 1, 3)   # (hg,q,hsub,bj)

        t8 = (t0t[h0:h0 + HL].reshape(HG, 8, Q, Q)
              .transpose(0, 2, 1, 3))                            # (hg,s,hsub,t)
        v8 = (vpk[h0:h0 + HL].reshape(HG, 8, Q, 2 * N)
              .transpose(0, 2, 1, 3))                            # (hg,q,hsub,2n)
        p8 = (ppk[hp0:hp0 + HP].reshape(HG, 4, 2 * N, Q)
              .transpose(0, 2, 1, 3))                            # (hg,2n,hp4,t)
        m8 = (mpk[hp0:hp0 + HP].reshape(HG, 4, 2 * N, Q)
              .transpose(0, 2, 1, 3))

        per_core.append(dict(
            u_t=np.ascontiguousarray(u_t8).astype(F16),
            t0t=np.ascontiguousarray(t8).astype(F16),
            vpk=np.ascontiguousarray(v8).astype(F16),
            ppk=np.ascontiguousarray(p8).astype(F16),
            mpk=np.ascontiguousarray(m8).astype(F16),
            wr=wr, wi=wi,
        ))

    # outw_m[m][p][d*128+t] = out_w.T[d*96+p, m*128+t]  (per-m contiguous)
    outw_m = (out_w.T.reshape(P, HL, H // 128, 128)
              .transpose(2, 1, 0, 3).reshape(H // 128, HL, P * 128))
    outw_m = np.ascontiguousarray(outw_m).astype(F16)
    outb = np.ascontiguousarray(
        out_b[:, 0].reshape(H // 128, 128).T).astype(np.float32)  # (128, 6)
    return per_core, outw_m, outb


def _build_nc():
    from concourse import bacc
    import concourse.mybir as mybir
    from concourse.tile import TileContext

    f32 = mybir.dt.float32
    f16 = mybir.dt.float16
    ACT = mybir.ActivationFunctionType
    ALU = mybir.AluOpType

    nc = bacc.Bacc()
    u_t = nc.dram_tensor("u_t", [HG, Q, 8 * BJ], f16, kind="ExternalInput")
    t0t = nc.dram_tensor("t0t", [HG, Q, 8 * Q], f16, kind="ExternalInput")
    vpk = nc.dram_tensor("vpk", [HG, Q, 8 * 2 * N], f16, kind="ExternalInput")
    ppk = nc.dram_tensor("ppk", [HG, 2 * N, 4 * Q], f16, kind="ExternalInput")
    mpk = nc.dram_tensor("mpk", [HG, 2 * N, 4 * Q], f16, kind="ExternalInput")
    wr_d = nc.dram_tensor("wr", [128, HP * B], f16, kind="ExternalInput")
    wi_d = nc.dram_tensor("wi", [128, HP * B], f16, kind="ExternalInput")
    outwT = nc.dram_tensor("outwT", [H // 128, HL, P * 128], f16,
                           kind="ExternalInput")
    outb = nc.dram_tensor("outb", [128, H // 128], f32, kind="ExternalInput")
    y_out = nc.dram_tensor("y", [H, L], f32, kind="ExternalOutput")

    with TileContext(nc) as tc:
        with (
            tc.tile_pool(name="dram", bufs=1, space="DRAM") as dpool,
            tc.tile_pool(name="scan", bufs=1) as spool,
            tc.tile_pool(name="uin", bufs=3) as upool,
            tc.tile_pool(name="wts", bufs=3) as wpool,
            tc.tile_pool(name="tmp", bufs=4) as tpool,
            tc.tile_pool(name="gout", bufs=4) as gpool,
            tc.tile_pool(name="outph", bufs=1) as opool,
            tc.tile_pool(name="ostr", bufs=3) as ospool,
            tc.tile_pool(name="ps_s", bufs=2, space="PSUM") as ps_s,
            tc.tile_pool(name="ps_y", bufs=2, space="PSUM") as ps_y,
            tc.tile_pool(name="ps_o", bufs=4, space="PSUM") as ps_o,
        ):
            # payload layout [b, j, h_local, t]
            a2a_in = dpool.tile([B, J, HL, Q], f16)
            a2a_out = dpool.tile([B, J, HL, Q], f16)

            wr_t = spool.tile([128, HP * B], f16, tag="wr")
            wi_t = spool.tile([128, HP * B], f16, tag="wi")
            nc.sync.dma_start(wr_t[:], wr_d[:])
            nc.sync.dma_start(wi_t[:], wi_d[:])

            # scan buffers: free = (hp, j, b); partitions = hs*64+n.
            # per-hp slice [hp*128:(hp+1)*128] is contiguous with column
            # order (j,b) matching the conv psum partition order.
            Sre = spool.tile([128, HP * J * B], f16, tag="Sre")
            Sim = spool.tile([128, HP * J * B], f16, tag="Sim")
            Zre = spool.tile([128, HP * J * B], f16, tag="Zre")
            Zim = spool.tile([128, HP * J * B], f16, tag="Zim")

            def hjb(t):
                return t[:].rearrange("p (hp j b) -> p hp j b", hp=HP, j=J)

            # ---------------- phase 1: summaries ----------------
            for hg in range(HG):
                ut8 = upool.tile([Q, 8 * BJ], f16, tag="u")
                nc.sync.dma_start(ut8[:], u_t[hg])
                vt8 = wpool.tile([Q, 8 * 2 * N], f16, tag="v")
                nc.sync.dma_start(vt8[:], vpk[hg])
                for half in range(2):
                    ps = ps_s.tile([128, 512], f32)
                    for h4 in range(4):
                        hsub = half * 4 + h4
                        hs = hsub % 2
                        hp2 = h4 // 2          # pair within this psum tile
                        for ri in range(2):
                            nc.tensor.matmul(
                                ps[hs * 64:(hs + 1) * 64,
                                   hp2 * 256 + ri * 128:
                                   hp2 * 256 + ri * 128 + 128],
                                vt8[:, hsub * 128 + ri * 64:
                                    hsub * 128 + (ri + 1) * 64],
                                ut8[:, hsub * 128:(hsub + 1) * 128],
                                start=True, stop=True,
                            )
                    # copy 4 h of S into scan layout
                    hp_base = hg * 4 + half * 2
                    psv = ps[:].rearrange("p (hp2 ri j b) -> p hp2 ri j b",
                                          hp2=2, ri=2, j=J)
                    for ri, dstt in ((0, Sre), (1, Sim)):
                        dst = hjb(dstt)[:, hp_base:hp_base + 2, :, :]
                        nc.vector.tensor_copy(dst, psv[:, :, ri, :, :])

            # ---------------- scan (fp16, DVE + GpSimd) ----------------
            for g in range(NG):
                hp0 = g * GHP
                for eng, lo, hi in ((nc.vector, hp0, hp0 + SCAN_SPLIT),
                                    (nc.gpsimd, hp0 + SCAN_SPLIT, hp0 + GHP)):
                    nhp = hi - lo
                    if nhp <= 0:
                        continue

                    def sl(t, j, lo=lo, hi=hi):
                        return hjb(t)[:, lo:hi, j, :]

                    wrv = (wr_t[:].rearrange("p (hp b) -> p hp b", hp=HP)
                           [:, lo:hi, :])
                    wiv = (wi_t[:].rearrange("p (hp b) -> p hp b", hp=HP)
                           [:, lo:hi, :])
                    eng.memset(sl(Zre, 0), 0.0)
                    eng.memset(sl(Zim, 0), 0.0)
                    tag = f"s{g}{lo}"
                    for j in range(1, J):
                        zr_p, zi_p = sl(Zre, j - 1), sl(Zim, j - 1)
                        a = tpool.tile([128, nhp, B], f16, tag=tag + "a")
                        b_ = tpool.tile([128, nhp, B], f16, tag=tag + "b")
                        eng.tensor_mul(a[:], zr_p, wrv)
                        eng.tensor_mul(b_[:], zi_p, wiv)
                        eng.tensor_sub(a[:], a[:], b_[:])
                        eng.tensor_add(sl(Zre, j), a[:], sl(Sre, j - 1))
                        a2 = tpool.tile([128, nhp, B], f16, tag=tag + "c")
                        b2 = tpool.tile([128, nhp, B], f16, tag=tag + "d")
                        eng.tensor_mul(a2[:], zi_p, wrv)
                        eng.tensor_mul(b2[:], zr_p, wiv)
                        eng.tensor_add(a2[:], a2[:], b2[:])
                        eng.tensor_add(sl(Zim, j), a2[:], sl(Sim, j - 1))

            # ---------------- phase 2: intra + apply + gelu ----------------
            for hg in range(HG):
                tt8 = wpool.tile([Q, 8 * Q], f16, tag="t0")
                nc.sync.dma_start(tt8[:], t0t[hg])
                pp8 = wpool.tile([2 * N, 4 * Q], f16, tag="p")
                mm8 = wpool.tile([2 * N, 4 * Q], f16, tag="m")
                nc.sync.dma_start(pp8[:], ppk[hg])
                nc.sync.dma_start(mm8[:], mpk[hg])
                ut8 = upool.tile([Q, 8 * BJ], f16, tag="u")
                nc.sync.dma_start(ut8[:], u_t[hg])
                for half in range(2):
                    psy = ps_y.tile([128, 512], f32)
                    for h4 in range(4):
                        hsub = half * 4 + h4
                        hs = hsub % 2
                        hp_loc = half * 2 + h4 // 2      # 0..3 within hg
                        hp = hg * 4 + hp_loc
                        out_ap = psy[:, h4 * 128:(h4 + 1) * 128]
                        nc.tensor.matmul(
                            out_ap,
                            ut8[:, hsub * 128:(hsub + 1) * 128],
                            tt8[:, hsub * 128:(hsub + 1) * 128],
                            start=True, stop=False)
                        nc.tensor.matmul(
                            out_ap,
                            Zre[hs * 64:(hs + 1) * 64,
                                hp * 128:(hp + 1) * 128],
                            pp8[hs * 64:(hs + 1) * 64,
                                hp_loc * 128:(hp_loc + 1) * 128],
                            start=False, stop=False)
                        nc.tensor.matmul(
                            out_ap,
                            Zim[hs * 64:(hs + 1) * 64,
                                hp * 128:(hp + 1) * 128],
                            mm8[hs * 64:(hs + 1) * 64,
                                hp_loc * 128:(hp_loc + 1) * 128],
                            start=False, stop=True)
                    gt = gpool.tile([128, 512], f16, tag="g")
                    nc.scalar.activation(gt[:], psy[:], ACT.Gelu)
                    # partitions are (j,b) j-major; store 4 h
                    h_base = hg * 8 + half * 4
                    dst = (a2a_in[:, :, h_base:h_base + 4, :]
                           .rearrange("b j hh t -> j b hh t"))
                    nc.sync.dma_start(dst, gt[:])

            # ---------------- AllToAll ----------------
            nc.gpsimd.collective_compute(
                "AllToAll", ALU.bypass,
                replica_groups=[list(range(P))],
                ins=[a2a_in.opt()], outs=[a2a_out.opt()],
            )

            # ---------------- output linear ----------------
            MT = H // 128
            g_tiles = []
            for d in range(P):
                gt = opool.tile([HL, L], f16, tag=f"gf{d}")
                for jh in range(2):  # split across queues
                    nc.sync.dma_start(
                        gt[:].rearrange("hh (j t) -> hh j t", t=Q)
                        [:, jh * 8:(jh + 1) * 8, :],
                        a2a_out[d].rearrange("j hh t -> hh j t")
                        [:, jh * 8:(jh + 1) * 8, :])
                g_tiles.append(gt)
            ob_t = opool.tile([128, MT], f32, tag="ob")
            nc.sync.dma_start(ob_t[:], outb[:])

            FT = L // 512
            for m in range(MT):
                ow_m = ospool.tile([HL, P * 128], f16, tag="ow")
                nc.sync.dma_start(ow_m[:], outwT[m])
                for f in range(FT):
                    pso = ps_o.tile([128, 512], f32)
                    for k in range(P):
                        nc.tensor.matmul(
                            pso[:],
                            ow_m[:, k * 128:(k + 1) * 128],
                            g_tiles[k][:, f * 512:(f + 1) * 512],
                            start=(k == 0), stop=(k == P - 1),
                        )
                    yt = tpool.tile([128, 512], f32, tag="yo")
                    nc.scalar.activation(yt[:], pso[:], ACT.Identity,
                                         bias=ob_t[:, m:m + 1], scale=1.0)
                    nc.sync.dma_start(
                        y_out[m * 128:(m + 1) * 128, f * 512:(f + 1) * 512],
                        yt[:])

    nc.finalize()
    return nc


_CACHED = {}


def kernel(u, log_dt, Lambda, W, D, out_w, out_b, _trace=False, _trace_cores=None):
    from concourse.bass_utils import run_bass_kernel_spmd

    per_core, outwT, outb = _host_precompute(
        np.asarray(u), np.asarray(log_dt), np.asarray(Lambda), np.asarray(W),
        np.asarray(D), np.asarray(out_w), np.asarray(out_b))

    if "nc" not in _CACHED:
        _CACHED["nc"] = _build_nc()
    nc = _CACHED["nc"]

    in_maps = []
    for c in range(P):
        m = dict(per_core[c])
        m["outwT"] = outwT
        m["outb"] = outb
        in_maps.append(m)

    kw = {}
    if _trace:
        kw = dict(trace=True, trace_cores=(_trace_cores or [0]))
    res = run_bass_kernel_spmd(nc, in_maps, core_ids=list(range(P)), **kw)
    out = np.stack([res.results[c]["y"] for c in range(P)])  # (B, H, L)
    if _trace:
        kernel._last_exec_time_ns = res.exec_time_ns
        kernel._last_result = res
    return out
